# revision 14
# baseline (speedup 1.0000x reference)
"""Trainium2 Bass kernel for nn_CliffordFourierHead (CGENN-style Clifford net).

Network (per reference): B=1024, IN=256, HID=512, OUT=128, Cl(3,0), 8 blades.
  fcgp1 -> MVSiLU -> channel-wise steerable GP -> MVSiLU -> fcgp2

Strategy (v3):
  - Pure batch data-parallelism over 8 NeuronCores (128 batch rows each).
  - Channels on partitions, batch on free dim; an activation is 8 blade
    planes packed into one [128, 8*128] SBUF tile per channel-tile.
  - All weights are packed HOST-SIDE into their exact SBUF layout
    [128, C] so every weight load is one fully-contiguous DMA (the v2
    strided loads generated ~36k tiny DMA packets).
  - Weight DMAs for stages 1+3 issue at t=0; stage-5 weights stream in
    while phases 3-4 compute (SBUF arena reuse via pool scoping).
  - PSUM per group is 2 banks (blades 0-3 | 4-7) instead of 4, halving
    evacuation / square instruction counts and letting 4 groups be in
    flight across the 8 banks.
  - Linear-term matmuls for a whole phase are emitted before the
    geometric-product matmuls so TensorE has work while DVE/Act build
    normalizations and product tiles.
  - Mega product tile Q[i,k] = x_i * xr_k is built split across DVE
    (i 0-5) and GpSimd (i 6-7) to offload the otherwise-idle engine.
  - Normalization chain runs fully in fp16 (4x DVE mode for the affine
    step, no fp32->fp16 casts).
  - Negative Cayley signs use on-chip negated weight copies (DVE 4x).

Self-contained: shapes and the Cl(3,0) Cayley table are derived inline.
"""

import contextlib
import math

import numpy as np

NCORES = 8
B, NIN, HID, NOUT = 1024, 256, 512, 128
BC = B // NCORES  # 128 batch rows per core
NB = 8
KT_IN, KT_HID = NIN // 128, HID // 128  # 2, 4
MT_IN, MT_HID, MT_OUT = NIN // 128, HID // 128, NOUT // 128  # 2, 4, 1
GRADE_SLICES = [(0, 1), (1, 4), (4, 7), (7, 8)]
EPS = 1e-6
ISQ2 = 1.0 / math.sqrt(2.0)


def _build_cayley():
    masks = sorted(range(NB), key=lambda m: (bin(m).count("1"), m))
    pos = {m: i for i, m in enumerate(masks)}
    cay = np.zeros((NB, NB, NB), dtype=np.float32)
    for i, mi in enumerate(masks):
        for k, mk in enumerate(masks):
            a, s = mi >> 1, 0
            while a:
                s += bin(a & mk).count("1")
                a >>= 1
            cay[i, pos[mi ^ mk], k] = -1.0 if (s & 1) else 1.0
    triples = []
    for gi in range(4):
        for gj in range(4):
            for gk in range(4):
                (i0, i1), (j0, j1), (k0, k1) = (
                    GRADE_SLICES[gi], GRADE_SLICES[gj], GRADE_SLICES[gk])
                if np.any(cay[i0:i1, j0:j1, k0:k1] != 0):
                    triples.append((gi, gj, gk))
    return cay, triples


CAY, TRIPLES = _build_cayley()
NPATHS = len(TRIPLES)  # 20

# Per triple t: {j: [(i, k, sign), ...]}
TRIPLE_TERMS = []
for t, (gi, gj, gk) in enumerate(TRIPLES):
    (i0, i1), (k0, k1) = GRADE_SLICES[gi], GRADE_SLICES[gk]
    d = {}
    for i in range(i0, i1):
        for k in range(k0, k1):
            j = int(np.nonzero(CAY[i, :, k])[0][0])
            if GRADE_SLICES[gj][0] <= j < GRADE_SLICES[gj][1]:
                d.setdefault(j, []).append((i, k, float(CAY[i, j, k])))
    TRIPLE_TERMS.append(d)


def _build_term_sets():
    """Per triple: matmul term-sets (j0, L, plane0, plane_step, sign).

    A term-set is a run of consecutive output blades j0..j0+L-1, one product
    plane each, uniform sign, arithmetic plane offsets (plane = i*8+k) ->
    a single matmul with a strided rhs plane selection.
    """
    all_sets = []
    for t in range(NPATHS):
        terms = []
        for j, lst in TRIPLE_TERMS[t].items():
            for (i, k, s) in lst:
                terms.append((j, i * 8 + k, s))
        sets = []
        for sgn in (1.0, -1.0):
            pool = sorted(x for x in terms if x[2] == sgn)
            while pool:
                j0, o0, _ = pool.pop(0)
                run = [(j0, o0)]
                step = None
                while True:
                    pick = None
                    for c in pool:
                        if c[0] != run[-1][0] + 1:
                            continue
                        st = c[1] - run[-1][1]
                        if step is None or st == step:
                            pick, pstep = c, st
                            break
                    if pick is None:
                        break
                    step = pstep
                    pool.remove(pick)
                    run.append((pick[0], pick[1]))
                sets.append((run[0][0], len(run), run[0][1], step or 0, sgn))
        all_sets.append(sets)
    return all_sets


TERM_SETS = _build_term_sets()
NEG_TRIPLES = sorted({t for t in range(NPATHS)
                      if any(s[4] < 0 for s in TERM_SETS[t])})
NEG_SLOT = {t: n for n, t in enumerate(NEG_TRIPLES)}
NNEG = len(NEG_TRIPLES)
NEG_RUNS = []
_i = 0
while _i < NNEG:
    _j = _i
    while _j + 1 < NNEG and NEG_TRIPLES[_j + 1] == NEG_TRIPLES[_j] + 1:
        _j += 1
    NEG_RUNS.append((NEG_TRIPLES[_i], _j - _i + 1))
    _i = _j + 1

GP_SETS_BY_GRADE = {g: [(t, s) for t in range(NPATHS)
                        if TRIPLES[t][1] == g
                        for s in TERM_SETS[t]]
                    for g in range(4)}
NSETS_G = [len(GP_SETS_BY_GRADE[g]) for g in range(4)]  # [8, 16, 16, 8]


# ----------------------------------------------------------------------------
# Host-side prep: pack everything into exact SBUF layouts [128, C]
# ----------------------------------------------------------------------------
def prep_in_maps(inputs):
    f16, f32 = np.float16, np.float32

    def pack_lin(w, scale=1.0):
        m, n, _ = np.asarray(w).shape
        kt = n // 128
        wt = np.asarray(w, f32).transpose(1, 2, 0) * scale  # [n, 4, m]
        wt = wt.reshape(kt, 128, 4, m).transpose(1, 0, 2, 3)
        return np.ascontiguousarray(wt.reshape(128, kt * 4 * m)).astype(f16)

    def pack_gp(w, scale):
        m, n, _ = np.asarray(w).shape
        kt = n // 128
        wt = np.asarray(w, f32).transpose(1, 2, 0) * scale  # [n, 20, m]
        wt = wt.reshape(kt, 128, NPATHS, m).transpose(1, 0, 2, 3)
        return np.ascontiguousarray(
            wt.reshape(128, kt * NPATHS * m)).astype(f16)

    def sig(a):
        return 1.0 / (1.0 + np.exp(-np.asarray(a, f32)))

    x = np.asarray(inputs["x"], f32)

    c = {}
    c["lr1w"] = pack_lin(inputs["lr1_w"])
    c["ll1w"] = pack_lin(inputs["ll1_w"], ISQ2)
    c["lrgw"] = pack_lin(inputs["lrg_w"])
    c["llgw"] = pack_lin(inputs["llg_w"], ISQ2)
    c["lr2w"] = pack_lin(inputs["lr2_w"])
    c["ll2w"] = pack_lin(inputs["ll2_w"], ISQ2)
    c["w1w"] = pack_gp(inputs["w1"], ISQ2)
    c["w2w"] = pack_gp(inputs["w2"], ISQ2)

    # channel-wise GP weights as diagonal matrices, packed [128, ct*20*128]
    wg = np.asarray(inputs["wg"], f32) * ISQ2  # [HID, 20]
    dwg = np.zeros((128, MT_HID, NPATHS, 128), f32)
    p = np.arange(128)
    for ct in range(MT_HID):
        for t in range(NPATHS):
            dwg[p, ct, t, p] = wg[ct * 128:(ct + 1) * 128, t]
    c["dwg"] = np.ascontiguousarray(
        dwg.reshape(128, MT_HID * NPATHS * 128)).astype(f16)

    cols = []  # [128, w] blocks, order must match device-side offsets

    def addp(arr):
        cols.append(np.asarray(arr, f32).reshape(128, -1))

    for nm, a, kt in (("n1", inputs["n1_a"], KT_IN),
                      ("ng", inputs["ng_a"], KT_HID),
                      ("n2", inputs["n2_a"], KT_HID)):
        sa = sig(a).reshape(kt, 128, 4)
        cb = (1.0 + EPS) - sa
        for u in range(kt):
            addp(sa[u])
            addp(cb[u])
    aa = np.asarray(inputs["act_a"], f32).reshape(MT_HID, 128, 4)
    ab = np.asarray(inputs["act_b"], f32).reshape(MT_HID, 128, 4)
    for u in range(MT_HID):
        addp(aa[u])
        addp(ab[u])
    addp((np.asarray(inputs["ll1_b"], f32) * ISQ2).reshape(MT_HID, 128).T)
    addp((np.asarray(inputs["llg_b"], f32) * ISQ2).reshape(MT_HID, 128).T)
    addp((np.asarray(inputs["ll2_b"], f32) * ISQ2).reshape(MT_OUT, 128).T)
    c["prm"] = np.ascontiguousarray(np.concatenate(cols, axis=1))

    in_maps = []
    for cid in range(NCORES):
        xc = x[cid * BC:(cid + 1) * BC]  # [BC, 256, 8]
        xt = np.transpose(xc, (1, 2, 0)).reshape(KT_IN, 128, NB, BC)
        xt = xt.transpose(1, 0, 2, 3).reshape(128, KT_IN * NB * BC)
        m = dict(c)
        m["xT"] = np.ascontiguousarray(xt).astype(f16)
        in_maps.append(m)
    return in_maps


def assemble(results):
    out = np.empty((B, NOUT, NB), np.float32)
    for cid in range(NCORES):
        od = np.asarray(results[cid]["outd"]).reshape(128, NB, BC)
        out[cid * BC:(cid + 1) * BC] = od.transpose(2, 0, 1)
    return out


# ----------------------------------------------------------------------------
# Device program (identical on all 8 cores)
# ----------------------------------------------------------------------------
def build_program():
    import concourse.mybir as mybir
    import concourse.tile as tile
    from concourse import bacc

    dt = mybir.dt
    AF = mybir.ActivationFunctionType
    OP = mybir.AluOpType

    nc = bacc.Bacc("TRN2", target_bir_lowering=False, debug=False,
                   num_devices=NCORES)

    def din(name, cols, dtype=dt.float16):
        return nc.dram_tensor(name, [128, cols], dtype,
                              kind="ExternalInput").ap()

    xT = din("xT", KT_IN * NB * BC)
    lr1w = din("lr1w", KT_IN * 4 * NIN)
    ll1w = din("ll1w", KT_IN * 4 * HID)
    w1w = din("w1w", KT_IN * NPATHS * HID)
    lrgw = din("lrgw", KT_HID * 4 * HID)
    llgw = din("llgw", KT_HID * 4 * HID)
    dwg = din("dwg", MT_HID * NPATHS * 128)
    lr2w = din("lr2w", KT_HID * 4 * HID)
    w2w = din("w2w", KT_HID * NPATHS * NOUT)
    ll2w = din("ll2w", KT_HID * 4 * NOUT)
    prm = din("prm", 121, dt.float32)
    outd = nc.dram_tensor("outd", [128, NB * BC], dt.float32,
                          kind="ExternalOutput").ap()

    with tile.TileContext(nc) as tc:
        top = contextlib.ExitStack()
        with top:
            ppool = top.enter_context(tc.tile_pool(name="params", bufs=1))
            hpool = top.enter_context(tc.tile_pool(name="hacts", bufs=1))
            qpool = top.enter_context(tc.tile_pool(name="q", bufs=2))
            npool = top.enter_context(tc.tile_pool(name="nsc", bufs=2))
            rawpool = top.enter_context(tc.tile_pool(name="raw", bufs=2))
            pspool = top.enter_context(
                tc.tile_pool(name="psum", bufs=8, space="PSUM"))

            prmt = ppool.tile([128, 121], dt.float32, tag="prm", name="prm")
            nc.sync.dma_start(prmt[:], prm)
            PN1, PNG, PN2, PACT, PB1, PBG, PB2 = 0, 16, 48, 80, 112, 116, 120

            n1sat = {u: prmt[:, PN1 + 8 * u:PN1 + 8 * u + 4]
                     for u in range(KT_IN)}
            n1cbt = {u: prmt[:, PN1 + 8 * u + 4:PN1 + 8 * u + 8]
                     for u in range(KT_IN)}
            ngsat = {u: prmt[:, PNG + 8 * u:PNG + 8 * u + 4]
                     for u in range(KT_HID)}
            ngcbt = {u: prmt[:, PNG + 8 * u + 4:PNG + 8 * u + 8]
                     for u in range(KT_HID)}
            n2sat = {u: prmt[:, PN2 + 8 * u:PN2 + 8 * u + 4]
                     for u in range(KT_HID)}
            n2cbt = {u: prmt[:, PN2 + 8 * u + 4:PN2 + 8 * u + 8]
                     for u in range(KT_HID)}
            actat = {u: prmt[:, PACT + 8 * u:PACT + 8 * u + 4]
                     for u in range(MT_HID)}
            actbt = {u: prmt[:, PACT + 8 * u + 4:PACT + 8 * u + 8]
                     for u in range(MT_HID)}
            b1t = {u: prmt[:, PB1 + u:PB1 + u + 1] for u in range(MT_HID)}
            bgt = {u: prmt[:, PBG + u:PBG + u + 1] for u in range(MT_HID)}
            b2t = {0: prmt[:, PB2:PB2 + 1]}

            # ---------------- shared helpers --------------------------------
            class Em:
                """start on first / stop on last matmul per (bank, class)."""

                def __init__(self, totals):
                    self.totals = dict(totals)
                    self.seen = {}

                def mm(self, key, dst, lhs, rhs):
                    i = self.seen.get(key, 0)
                    nc.tensor.matmul(dst, lhs, rhs, start=(i == 0),
                                     stop=(i == self.totals[key] - 1))
                    self.seen[key] = i + 1

                def done(self):
                    assert self.seen == self.totals, (self.seen, self.totals)

            def lin_totals(nkt):
                return {"A": 2 * nkt, "B": 2 * nkt}

            def h_totals(nkt_lin, nkt_gp):
                return {"A": 2 * nkt_lin + nkt_gp * (NSETS_G[0]
                                                     + NSETS_G[1]),
                        "B": 2 * nkt_lin + nkt_gp * (NSETS_G[2]
                                                     + NSETS_G[3])}

            def alloc_ps(nm):
                a = pspool.tile([128, 4 * BC], dt.float32, tag="ps",
                                name=f"psA_{nm}")
                b = pspool.tile([128, 4 * BC], dt.float32, tag="ps",
                                name=f"psB_{nm}")
                return a, b

            def emit_lin(em, psA, psB, wsl, xs, nkt, mt):
                for kt in range(nkt):
                    xk = xs(kt)
                    em.mm("A", psA[:, 0:BC], wsl(kt, 0, mt),
                          xk[:, 0:BC])
                    em.mm("A", psA[:, BC:4 * BC], wsl(kt, 1, mt),
                          xk[:, BC:4 * BC])
                    em.mm("B", psB[:, 0:3 * BC], wsl(kt, 2, mt),
                          xk[:, 4 * BC:7 * BC])
                    em.mm("B", psB[:, 3 * BC:4 * BC], wsl(kt, 3, mt),
                          xk[:, 7 * BC:8 * BC])

            def plane_sel(qpl, o0, L, st):
                if L == 1:
                    return qpl[:, o0:o0 + 1, :]
                last = o0 + st * (L - 1)
                stop = last + 1 if st > 0 else (last - 1 if last >= 1 else None)
                return qpl[:, o0:stop:st, :]

            def emit_gp(em, psA, psB, wsl, wsln, qpl, kt, mt):
                for g in range(4):
                    for (t, (j0, L, o0, st, sgn)) in GP_SETS_BY_GRADE[g]:
                        lhs = (wsl if sgn > 0 else wsln)(kt, t, mt)
                        rhs = plane_sel(qpl, o0, L, st)
                        if g == 0:
                            key, dst = "A", psA[:, 0:BC]
                        elif g == 1:
                            key = "A"
                            dst = psA[:, j0 * BC:(j0 + L) * BC]
                        elif g == 2:
                            r0 = j0 - 4
                            key = "B"
                            dst = psB[:, r0 * BC:(r0 + L) * BC]
                        else:
                            key, dst = "B", psB[:, 3 * BC:4 * BC]
                        em.mm(key, dst, lhs, rhs)

            def build_q(xs, xrs, nm):
                """Q[i*8+k] = x_i * xr_k, [128, 8192] fp16.
                DVE builds i 0-5, GpSimd builds i 6-7."""
                q = qpool.tile([128, 64 * BC], dt.float16, tag="Q",
                               name=f"Q_{nm}")
                qv = q[:].rearrange("p (i k b) -> p i k b", i=8, k=8)
                xrb = xrs.rearrange("p (u k b) -> p u k b", u=1, k=8)
                for (i0, i1, eng) in ((0, 4, nc.vector), (4, 6, nc.vector),
                                      (6, 8, nc.gpsimd)):
                    ni = i1 - i0
                    a = xs[:, i0 * BC:i1 * BC].rearrange(
                        "p (i u b) -> p i u b", i=ni,
                        u=1).broadcast_to([128, ni, 8, BC])
                    bb = xrb.broadcast_to([128, ni, 8, BC])
                    eng.tensor_mul(qv[:, i0:i1], a, bb)
                return q

            def evac(psA, psB, raw, bias=None):
                if bias is None:
                    nc.scalar.copy(raw[:, 0:4 * BC], psA[:])
                else:
                    nc.scalar.activation(raw[:, 0:BC], psA[:, 0:BC],
                                         AF.Identity, bias=bias)
                    nc.scalar.copy(raw[:, BC:4 * BC], psA[:, BC:4 * BC])
                nc.scalar.copy(raw[:, 4 * BC:], psB[:])

            def gate_op(out, raw, gate, op):
                """out[j] = raw[j] (op) gate[grade(j)] for 8 blade planes."""
                nc.vector.tensor_tensor(out[:, 0:BC], raw[:, 0:BC],
                                        gate[:, 0:BC], op)
                bb = gate[:, BC:3 * BC].rearrange(
                    "p (g u b) -> p g u b", g=2,
                    u=1).broadcast_to([128, 2, 3, BC])
                nc.vector.tensor_tensor(
                    out[:, BC:7 * BC].rearrange(
                        "p (g i b) -> p g i b", g=2, i=3),
                    raw[:, BC:7 * BC].rearrange(
                        "p (g i b) -> p g i b", g=2, i=3), bb, op)
                nc.vector.tensor_tensor(out[:, 7 * BC:], raw[:, 7 * BC:],
                                        gate[:, 3 * BC:], op)

            def normalize(psA, psB, raw, out, sat, cbt):
                sqw = npool.tile([128, 8 * BC], dt.float16, tag="sqw",
                                 name="sqw")
                nc.scalar.activation(sqw[:, 0:4 * BC], psA[:], AF.Square)
                nc.scalar.activation(sqw[:, 4 * BC:], psB[:], AF.Square)
                qw = npool.tile([128, 2 * BC], dt.float16, tag="qw",
                                name="qw")
                sqp = sqw[:, BC:7 * BC].rearrange(
                    "p (g i b) -> p g i b", g=2, i=3)
                qp = qw[:].rearrange("p (g b) -> p g b", g=2)
                nc.vector.tensor_add(qp, sqp[:, :, 0, :], sqp[:, :, 1, :])
                nc.vector.tensor_add(qp, qp, sqp[:, :, 2, :])
                nrmw = npool.tile([128, 4 * BC], dt.float16, tag="nrmw",
                                  name="nrmw")
                nc.scalar.activation(nrmw[:, BC:3 * BC], qw[:], AF.Sqrt)
                s07 = sqw[:].rearrange("p (i b) -> p i b", i=8)[:, 0:8:7, :]
                n03 = nrmw[:].rearrange("p (i b) -> p i b", i=4)[:, 0:4:3, :]
                nc.scalar.activation(n03, s07, AF.Sqrt)
                dw = npool.tile([128, 4 * BC], dt.float32, tag="dw",
                                name="dw")
                for g in range(4):
                    nc.vector.tensor_scalar(dw[:, g * BC:(g + 1) * BC],
                                            nrmw[:, g * BC:(g + 1) * BC],
                                            sat[:, g:g + 1], cbt[:, g:g + 1],
                                            OP.mult, OP.add)
                rw = npool.tile([128, 4 * BC], dt.float32, tag="rw",
                                name="rw")
                nc.vector.reciprocal_approx_fast(rw[:], dw[:])
                r16 = npool.tile([128, 4 * BC], dt.float16, tag="r16",
                                 name="r16")
                nc.vector.tensor_copy(r16[:], rw[:])
                gate_op(out, raw, r16, OP.mult)

            def mv_silu(psA, psB, raw, out, at, bt):
                sqw = npool.tile([128, 7 * BC], dt.float16, tag="sq7",
                                 name="sq7")
                nc.scalar.activation(sqw[:, 0:3 * BC], psA[:, BC:4 * BC],
                                     AF.Square)
                nc.scalar.activation(sqw[:, 3 * BC:7 * BC], psB[:],
                                     AF.Square)
                q12 = npool.tile([128, 2 * BC], dt.float16, tag="q12",
                                 name="q12")
                sqp = sqw[:, 0:6 * BC].rearrange(
                    "p (g i b) -> p g i b", g=2, i=3)
                qp = q12[:].rearrange("p (g b) -> p g b", g=2)
                nc.vector.tensor_add(qp, sqp[:, :, 0, :], sqp[:, :, 1, :])
                nc.vector.tensor_add(qp, qp, sqp[:, :, 2, :])
                gw = npool.tile([128, 4 * BC], dt.float16, tag="gw",
                                name="gw")
                invs = [raw[:, 0:BC], q12[:, 0:BC], q12[:, BC:],
                        sqw[:, 6 * BC:7 * BC]]
                for g in range(4):
                    nc.scalar.activation(gw[:, g * BC:(g + 1) * BC],
                                         invs[g], AF.Sigmoid,
                                         bias=bt[:, g:g + 1],
                                         scale=at[:, g:g + 1])
                gate_op(out, raw, gw, OP.mult)

            def load_w(pool, name, src, cols, eng=None):
                t = pool.tile([128, cols], dt.float16, tag=name, name=name)
                (eng or nc.sync).dma_start(t[:], src)
                return t

            def lin_sl(t, mtot):
                def sl(kt, g, mt):
                    base = (kt * 4 + g) * mtot + mt * 128
                    return t[:, base:base + 128]
                return sl

            def gp_sl(t, tn, mtot):
                def sl(kt, tt, mt):
                    base = (kt * NPATHS + tt) * mtot + mt * 128
                    return t[:, base:base + 128]

                def sln(kt, tt, mt):
                    base = (kt * NNEG + NEG_SLOT[tt]) * mtot + mt * 128
                    return tn[:, base:base + 128]
                return sl, sln

            def negate_gp(pool, name, t, nkt, mtot):
                tn = pool.tile([128, nkt * NNEG * mtot], dt.float16,
                               tag=name, name=name)
                for kt in range(nkt):
                    for (t0, ln) in NEG_RUNS:
                        sb = (kt * NPATHS + t0) * mtot
                        db = (kt * NNEG + NEG_SLOT[t0]) * mtot
                        nc.vector.tensor_scalar_mul(
                            tn[:, db:db + ln * mtot],
                            t[:, sb:sb + ln * mtot], -1.0)
                return tn

            # ================= program ======================================
            with tc.tile_pool(name="wB", bufs=1) as wB:
                H = {}
                with tc.tile_pool(name="wA", bufs=1) as wA, \
                     tc.tile_pool(name="xacts", bufs=1) as xpool:
                    # stage-1 inputs first on the sync queue, bulk weights
                    # for stages 1+3 on other engines' queues in parallel
                    xt_ = xpool.tile([128, KT_IN * NB * BC], dt.float16,
                                     tag="X", name="X")
                    nc.sync.dma_start(xt_[:], xT)

                    def xs(kt):
                        return xt_[:, kt * NB * BC:(kt + 1) * NB * BC]

                    lr1t_ = load_w(wA, "lr1w", lr1w, KT_IN * 4 * NIN)
                    ll1t_ = load_w(wA, "ll1w", ll1w, KT_IN * 4 * HID,
                                   eng=nc.scalar)
                    w1t_ = load_w(wA, "w1w", w1w, KT_IN * NPATHS * HID,
                                  eng=nc.scalar)
                    lrgt_ = load_w(wB, "lrgw", lrgw, KT_HID * 4 * HID,
                                   eng=nc.gpsimd)
                    llgt_ = load_w(wB, "llgw", llgw, KT_HID * 4 * HID,
                                   eng=nc.gpsimd)
                    lrgt = lin_sl(lrgt_, HID)
                    llgt = lin_sl(llgt_, HID)
                    lr1t = lin_sl(lr1t_, NIN)
                    ll1t = lin_sl(ll1t_, HID)

                    xrt_ = xpool.tile([128, MT_IN * NB * BC], dt.float16,
                                      tag="XR", name="XR")

                    def xrs(mt):
                        return xrt_[:, mt * NB * BC:(mt + 1) * NB * BC]

                    # --- phase 1: xr = normalization(lr1(x)) ---------------
                    lr1ps = {}
                    for mt in range(MT_IN):
                        psA, psB = alloc_ps(f"lr1_{mt}")
                        em = Em(lin_totals(KT_IN))
                        emit_lin(em, psA, psB, lr1t, xs, KT_IN, mt)
                        em.done()
                        lr1ps[mt] = (psA, psB)

                    # h psum groups; emit ll1 lin matmuls for all mt first
                    hps, hem = {}, {}
                    for mt in range(MT_HID):
                        psA, psB = alloc_ps(f"h_{mt}")
                        em = Em(h_totals(KT_IN, KT_IN))
                        emit_lin(em, psA, psB, ll1t, xs, KT_IN, mt)
                        hps[mt], hem[mt] = (psA, psB), em

                    # normalize chains + Q builds
                    Q = {}
                    for mt in range(MT_IN):
                        psA, psB = lr1ps[mt]
                        raw = rawpool.tile([128, NB * BC], dt.float16,
                                           tag="raw", name=f"rawxr_{mt}")
                        evac(psA, psB, raw)
                        normalize(psA, psB, raw, xrs(mt),
                                  n1sat[mt], n1cbt[mt])
                        Q[mt] = build_q(xs(mt), xrs(mt), f"x{mt}")

                    # negated w1 copies (DVE, after the latency-critical
                    # normalize/Q ops so the w1w DMA wait can't stall them)
                    w1nt_ = negate_gp(wA, "w1n", w1t_, KT_IN, HID)
                    w1t, w1nt = gp_sl(w1t_, w1nt_, HID)

                    # GP matmuls + silu chains
                    for mt in range(MT_HID):
                        psA, psB = hps[mt]
                        for kt in range(KT_IN):
                            qpl = Q[kt][:].rearrange(
                                "p (pl b) -> p pl b", pl=64)
                            emit_gp(hem[mt], psA, psB, w1t, w1nt, qpl,
                                    kt, mt)
                        hem[mt].done()
                        raw = rawpool.tile([128, NB * BC], dt.float16,
                                           tag="raw", name=f"rawh_{mt}")
                        evac(psA, psB, raw, bias=b1t[mt])
                        h = hpool.tile([128, NB * BC], dt.float16,
                                       tag=f"H_{mt}", name=f"H_{mt}")
                        mv_silu(psA, psB, raw, h, actat[mt], actbt[mt])
                        H[mt] = h

                # ---- phases 3-6 ---------------------------------------
                def hsl(kt):
                    return H[kt][:]

                H2 = {}
                with tc.tile_pool(name="wC", bufs=1) as wC:
                    with tc.tile_pool(name="p34", bufs=1) as p34:
                        # dwg is needed early in phase 3; stage-5 weights
                        # stream behind it on the same queue
                        dwgt_ = load_w(p34, "dwg", dwg,
                                       MT_HID * NPATHS * 128)
                        lr2t_ = load_w(wC, "lr2w", lr2w, KT_HID * 4 * HID)
                        w2t_ = load_w(wC, "w2w", w2w,
                                      KT_HID * NPATHS * NOUT)
                        ll2t_ = load_w(wC, "ll2w", ll2w, KT_HID * 4 * NOUT)
                        lr2t = lin_sl(lr2t_, HID)
                        ll2t = lin_sl(ll2t_, NOUT)

                        hrt_ = p34.tile([128, MT_HID * NB * BC],
                                        dt.float16, tag="HR", name="HR")

                        def hrs(mt):
                            return hrt_[:, mt * NB * BC:(mt + 1) * NB * BC]

                        lrgps, h2ps, h2em = {}, {}, {}

                        def lrg_group(mt):
                            psA, psB = alloc_ps(f"lrg_{mt}")
                            em = Em(lin_totals(KT_HID))
                            emit_lin(em, psA, psB, lrgt, hsl, KT_HID, mt)
                            em.done()
                            lrgps[mt] = (psA, psB)

                        def h2_group(mt):
                            psA, psB = alloc_ps(f"h2_{mt}")
                            em = Em(h_totals(KT_HID, 1))
                            emit_lin(em, psA, psB, llgt, hsl, KT_HID, mt)
                            h2ps[mt], h2em[mt] = (psA, psB), em

                        def hr_chain(mt):
                            psA, psB = lrgps[mt]
                            raw = rawpool.tile([128, NB * BC], dt.float16,
                                               tag="raw",
                                               name=f"rawhr_{mt}")
                            evac(psA, psB, raw)
                            normalize(psA, psB, raw, hrs(mt),
                                      ngsat[mt], ngcbt[mt])

                        def dsl(kt, tt, ct):
                            base = (ct * NPATHS + tt) * 128
                            return dwgt_[:, base:base + 128]

                        Qg = {}

                        def cw_gp(mt):
                            q = build_q(hsl(mt), hrs(mt), f"g{mt}")
                            qpl = q[:].rearrange("p (pl b) -> p pl b", pl=64)
                            psA2, psB2 = h2ps[mt]
                            emit_gp(h2em[mt], psA2, psB2, dsl, dsln, qpl,
                                    0, mt)
                            h2em[mt].done()

                        # sqrt-table chains first, sigmoid chains last, GP
                        # emission interleaved so psum slot reuse never
                        # waits on a later tensor instruction
                        lrg_group(0)
                        lrg_group(1)
                        h2_group(0)
                        h2_group(1)
                        hr_chain(0)
                        hr_chain(1)
                        Qg[0] = build_q(hsl(0), hrs(0), "g0")
                        dwgnt_ = negate_gp(p34, "dwgn", dwgt_, MT_HID, 128)

                        def dsln(kt, tt, ct):
                            base = (ct * NNEG + NEG_SLOT[tt]) * 128
                            return dwgnt_[:, base:base + 128]

                        cw_gp0_q = Qg[0][:].rearrange(
                            "p (pl b) -> p pl b", pl=64)
                        emit_gp(h2em[0], h2ps[0][0], h2ps[0][1], dsl, dsln,
                                cw_gp0_q, 0, 0)
                        h2em[0].done()
                        cw_gp(1)
                        lrg_group(2)
                        hr_chain(2)
                        lrg_group(3)
                        hr_chain(3)
                        h2_group(2)
                        h2_group(3)
                        cw_gp(2)
                        cw_gp(3)

                        for mt in range(MT_HID):
                            psA2, psB2 = h2ps[mt]
                            raw2 = rawpool.tile([128, NB * BC], dt.float16,
                                                tag="raw",
                                                name=f"rawh2_{mt}")
                            evac(psA2, psB2, raw2, bias=bgt[mt])
                            h2 = hpool.tile([128, NB * BC], dt.float16,
                                            tag=f"H2_{mt}", name=f"H2_{mt}")
                            mv_silu(psA2, psB2, raw2, h2,
                                    actat[mt], actbt[mt])
                            H2[mt] = h2

                    # --- phases 5-6 ------------------------------------
                    with tc.tile_pool(name="p56", bufs=1) as p56:
                        w2nt_ = negate_gp(wC, "w2n", w2t_, KT_HID, NOUT)
                        w2t, w2nt = gp_sl(w2t_, w2nt_, NOUT)

                        def h2sl(kt):
                            return H2[kt][:]

                        hr2t_ = p56.tile([128, MT_HID * NB * BC],
                                         dt.float16, tag="HR2", name="HR2")

                        def hr2s(mt):
                            return hr2t_[:, mt * NB * BC:(mt + 1) * NB * BC]

                        lr2ps = {}
                        for mt in range(MT_HID):
                            psA, psB = alloc_ps(f"lr2_{mt}")
                            em = Em(lin_totals(KT_HID))
                            emit_lin(em, psA, psB, lr2t, h2sl, KT_HID, mt)
                            em.done()
                            lr2ps[mt] = (psA, psB)

                        psA, psB = alloc_ps("out")
                        em = Em(h_totals(KT_HID, KT_HID))
                        emit_lin(em, psA, psB, ll2t, h2sl, KT_HID, 0)

                        for mt in range(MT_HID):
                            pA, pB = lr2ps[mt]
                            raw = rawpool.tile([128, NB * BC], dt.float16,
                                               tag="raw", name=f"rawr2_{mt}")
                            evac(pA, pB, raw)
                            normalize(pA, pB, raw, hr2s(mt),
                                      n2sat[mt], n2cbt[mt])
                            q = build_q(h2sl(mt), hr2s(mt), f"o{mt}")
                            qpl = q[:].rearrange("p (pl b) -> p pl b", pl=64)
                            emit_gp(em, psA, psB, w2t, w2nt, qpl, mt, 0)
                        em.done()

                        outs = p56.tile([128, NB * BC], dt.float32,
                                        tag="outs", name="outs")
                        evac(psA, psB, outs, bias=b2t[0])
                        nc.sync.dma_start(outd, outs[:])

    nc.compile()
    return nc


_PROGRAM = None


def _get_program():
    global _PROGRAM
    if _PROGRAM is None:
        _PROGRAM = build_program()
    return _PROGRAM


def kernel(**inputs):
    from concourse.bass_utils import run_bass_kernel_spmd

    nc = _get_program()
    in_maps = prep_in_maps(inputs)
    res = run_bass_kernel_spmd(nc, in_maps, core_ids=list(range(NCORES)))
    return assemble(res.results)


if __name__ == "__main__":
    print("NEG_TRIPLES:", NEG_TRIPLES)
    print("sets per grade:", NSETS_G)
